# revision 21
# baseline (speedup 1.0000x reference)
"""ArcFace loss on 8 TRN2 NeuronCores, tensor-parallel over the class dim.

Reference computation (B=1024, D=512, C=100000):
    e = l2norm(embeddings); w = l2norm(weight)
    cos = clip(e @ w.T);  phi = cos(theta + m) with easy-margin fallback
    logits = S * (onehot*phi + (1-onehot)*cos);  loss = mean CE

Distribution: classes sharded 12500/core. Host pre-normalizes both e and w
(row L2) and ships fp8-e4m3 copies for the big matmul. Each core computes
only its partial sum-of-exp Z[p, bt] = sum_c exp(S*cos[b, c]) over its
12500-class shard (fp8 DoubleRow matmuls into PSUM, 2 per 128-batch x
512-class tile; per-bank fused drains) and DMAs the [128, 8] partial back.
The host sums the 8 partials and finishes with the exact-target terms
(1024 dot products on the already-normalized host copies) in f64:
    nll[b] = log(Z_b - exp(S*cos_t_b) + exp(S*phi_b)) - S*phi_b
    loss   = mean_b nll[b]
No collective, no device epilogue, no device target path.

The exp work is split across engines so the Scalar engine stays ahead of
the PE: 5 of 8 batch tiles per chunk use the exact Scalar-engine Exp
(accum_out fusion), the other 3 use a Schraudolph bit-trick exp on the
Vector engine (int32 mad + bitcast reinterpret, ~2.5%/element with a
near-zero-mean magic constant) reduced in a second DVE pass. No softmax
max-shift is needed: |cos|<=1 so S*cos in [-64, 64] and exp() stays
comfortably inside fp32 range; Z is a 12.5k-term sum, so the per-element
noise washes out far inside the loss tolerance.
"""

import math

import numpy as np
import ml_dtypes

import concourse.bass as bass
import concourse.tile as tile
from concourse import bacc, mybir
from concourse.bass_utils import run_bass_kernel_spmd

# problem shapes (hardcoded per spec)
B, D, C = 1024, 512, 100000
N_CORES = 8
CS = C // N_CORES            # 12500 classes per core
NBT = B // 128               # 8 batch tiles
NKT = D // 128               # 4 contraction tiles
CHUNK = 512                  # matmul free-dim chunk
N_CHUNKS = (CS + CHUNK - 1) // CHUNK   # 25 (last chunk 212 wide)

# arcface constants
S = 64.0
M = 0.5
COS_M = math.cos(M)
SIN_M = math.sin(M)
TH = math.cos(math.pi - M)
MM_ = math.sin(math.pi - M) * M
EPS = 1e-7

F32 = mybir.dt.float32
I32 = mybir.dt.int32
FP8 = mybir.dt.float8e4

# Schraudolph exp-approx constants (folding in the S logit scale):
#   exp(S*x) ~= bitcast_f32(int32(SCHRA_A * x + SCHRA_B))
SCHRA_A = S * (1 << 23) / math.log(2.0)
SCHRA_B = float(127 * (1 << 23) - 486411)   # near-zero-mean magic constant
DVE_B = (2, 5, 7)                           # batch tiles handled by DVE

_NC_CACHE = []


def _emit_body(nc, pools, params):
    singles, work, wtp, tiny, psump = pools
    wt, et, out_ext = params

    # ---- replicated fp8 embedding tiles (stationary operands) ----
    et_s = singles.tile([128, NKT, B], FP8, name="et_s")
    for k in range(NKT):
        nc.sync.dma_start(out=et_s[:, k, :], in_=et[k * 128:(k + 1) * 128, :])

    zcols = singles.tile([128, NBT, N_CHUNKS // 2 + 1], F32,
                         name="zcols")

    # warm the Exp activation table (~2.7us load) under the input DMAs
    warm = tiny.tile([128, 1], F32, name="warm")
    nc.vector.memset(warm, 0.0)
    nc.scalar.activation(out=warm, in_=warm,
                         func=mybir.ActivationFunctionType.Exp)

    def drain(ps_ap, b, g, use_dve, axis):
        # one fused transform+row-sum of a PSUM group
        if use_dve:
            nc.vector.tensor_scalar(
                out=ps_ap.bitcast(I32), in0=ps_ap,
                scalar1=SCHRA_A, scalar2=SCHRA_B,
                op0=mybir.AluOpType.mult,
                op1=mybir.AluOpType.add)
            nc.vector.tensor_reduce(
                out=zcols[:, b, g:g + 1], in_=ps_ap,
                axis=axis, op=mybir.AluOpType.add)
        else:
            nc.scalar.activation(
                out=ps_ap, in_=ps_ap,
                func=mybir.ActivationFunctionType.Exp,
                scale=S, accum_out=zcols[:, b, g:g + 1])

    # ---- main pipeline: chunk pairs, drained as 2-bank groups ----
    for cp in range(N_CHUNKS // 2):
        c0 = cp * 2 * CHUNK
        wt8c = wtp.tile([128, NKT, 2 * CHUNK], FP8, name="wt8c")
        for k in range(NKT):
            nc.sync.dma_start(out=wt8c[:, k, :],
                              in_=wt[k * 128:(k + 1) * 128, c0:c0 + 2 * CHUNK])

        for b in range(NBT):
            ps = psump.tile([128, 2, CHUNK], F32, name="ps")
            for h in range(2):
                nc.tensor.matmul(
                    out=ps[:, h, :],
                    lhsT=et_s[:, 0:2, b * 128:(b + 1) * 128],
                    rhs=wt8c[:, 0:2, h * CHUNK:(h + 1) * CHUNK],
                    start=True, stop=False,
                    perf_mode=mybir.MatmulPerfMode.DoubleRow)
                nc.tensor.matmul(
                    out=ps[:, h, :],
                    lhsT=et_s[:, 2:4, b * 128:(b + 1) * 128],
                    rhs=wt8c[:, 2:4, h * CHUNK:(h + 1) * CHUNK],
                    start=False, stop=True,
                    perf_mode=mybir.MatmulPerfMode.DoubleRow)
            drain(ps, b, cp, b in DVE_B, mybir.AxisListType.XY)

    # ---- 212-class tail chunk: per-bank drains ----
    ct = N_CHUNKS - 1
    c0 = ct * CHUNK
    cw = CS - c0
    wt8t = wtp.tile([128, NKT, CHUNK], FP8, name="wt8t")
    for k in range(NKT):
        nc.sync.dma_start(out=wt8t[:, k, :cw],
                          in_=wt[k * 128:(k + 1) * 128, c0:c0 + cw])
    for b in range(NBT):
        ps = psump.tile([128, 2, CHUNK], F32, name="ps")
        nc.tensor.matmul(
            out=ps[:, 0, :cw],
            lhsT=et_s[:, 0:2, b * 128:(b + 1) * 128],
            rhs=wt8t[:, 0:2, :cw],
            start=True, stop=False,
            perf_mode=mybir.MatmulPerfMode.DoubleRow)
        nc.tensor.matmul(
            out=ps[:, 0, :cw],
            lhsT=et_s[:, 2:4, b * 128:(b + 1) * 128],
            rhs=wt8t[:, 2:4, :cw],
            start=False, stop=True,
            perf_mode=mybir.MatmulPerfMode.DoubleRow)
        drain(ps[:, 0, :cw], b, N_CHUNKS // 2, b in DVE_B,
              mybir.AxisListType.X)

    # ---- fold the per-chunk partials and ship Z_partial home ----
    zloc = tiny.tile([128, NBT], F32, name="zloc")
    nc.vector.tensor_reduce(out=zloc, in_=zcols,
                            axis=mybir.AxisListType.X,
                            op=mybir.AluOpType.add)
    nc.sync.dma_start(out=out_ext[:, :], in_=zloc)


def _declare_params(nc):
    wt = nc.declare_dram_parameter("wt", [D, CS], FP8, isOutput=False)
    et = nc.declare_dram_parameter("et", [D, B], FP8, isOutput=False)
    out_ext = nc.declare_dram_parameter("out", [128, NBT], F32, isOutput=True)
    return (wt, et, out_ext)


def _make_pools(tc, bufs_mult=1):
    return (
        tc.tile_pool(name="singles", bufs=bufs_mult),
        tc.tile_pool(name="work", bufs=4),
        tc.tile_pool(name="wtp", bufs=3),
        tc.tile_pool(name="tiny", bufs=bufs_mult),
        tc.tile_pool(name="psum", bufs=4, space="PSUM"),
    )


def _build(finalize=True):
    nc = bacc.Bacc(num_devices=N_CORES)
    params = _declare_params(nc)
    with tile.TileContext(nc) as tc:
        p0, p1, p2, p3, p4 = _make_pools(tc)
        with p0 as singles, p1 as work, p2 as wtp, p3 as tiny, p4 as psump:
            _emit_body(nc, (singles, work, wtp, tiny, psump), params)
    if finalize:
        nc.finalize()
    return nc


def _get_nc():
    if not _NC_CACHE:
        _NC_CACHE.append(_build())
    return _NC_CACHE[0]


def _prep_inputs(embeddings, labels, weight):
    e = np.asarray(embeddings, dtype=np.float32)
    w = np.asarray(weight, dtype=np.float32)
    lab = np.asarray(labels).astype(np.int64)

    # host-side row L2 normalization (dtype/layout prep for the device matmul)
    en = e / np.maximum(np.sqrt((e * e).sum(axis=1, keepdims=True)), 1e-12)
    wn = w / np.maximum(np.sqrt((w * w).sum(axis=1, keepdims=True)), 1e-12)

    et8 = np.ascontiguousarray(en.T).astype(ml_dtypes.float8_e4m3)  # [D, B]
    wt8_full = wn.T.astype(ml_dtypes.float8_e4m3)                   # [D, C]

    in_maps = []
    for i in range(N_CORES):
        sl = slice(CS * i, CS * (i + 1))
        in_maps.append({
            "wt": np.ascontiguousarray(wt8_full[:, sl]),
            "et": et8,
        })

    # exact target-logit path (1024 rows) on host, f64
    cos_t = np.clip((en.astype(np.float64) * wn[lab].astype(np.float64))
                    .sum(axis=1), -1.0 + EPS, 1.0 - EPS)
    sin_t = np.sqrt(1.0 - cos_t * cos_t)
    phi = cos_t * COS_M - sin_t * SIN_M
    phi = np.where(cos_t > TH, phi, cos_t - MM_)
    st = S * phi
    tgt = {"st": st, "ect": np.exp(S * cos_t), "ept": np.exp(st)}
    return in_maps, tgt


def _finish(results, tgt):
    # z[p, bt] per core -> full Z[b], b = bt*128 + p
    z = np.zeros((128, NBT), np.float64)
    for r in results:
        z += np.asarray(r["out"], dtype=np.float64)
    zfull = z.T.reshape(B)
    zmod = zfull - tgt["ect"] + tgt["ept"]
    nll = np.log(zmod) - tgt["st"]
    return np.float32(nll.mean())


def kernel(embeddings, labels, weight):
    in_maps, tgt = _prep_inputs(embeddings, labels, weight)
    nc = _get_nc()
    res = run_bass_kernel_spmd(nc, in_maps, list(range(N_CORES)))
    return _finish(res.results, tgt)
